# revision 16
# baseline (speedup 1.0000x reference)
"""Expert-parallel MoE (soft routing) kernel for 8 TRN2 NeuronCores.

Problem (nn_EnhancedMixtureOfExperts): every expert processes the full batch,
outputs mixed by soft cluster probabilities.

    h1 = relu(x @ W1[e] + b1[e])      x:[B,D]  W1[e]:[D,H]
    h2 = relu(h1 @ W2[e] + b2[e])     W2[e]:[H,H2]
    y  = sigmoid(h2 @ W3[e] + b3[e])  W3[e]:[H2,1]
    out[b] = sum_e y[e,b] * probs[b,e]

Sharding: expert-parallel — core e computes expert e over the full batch.
x is transposed on the host to xT [D,B] so on-chip activations are stored
feature-on-partition ([feat,128-block] x [batch]) and every GEMM consumes
weights in their natural [in,out] layout as the stationary operand.
The weighted combine is 8*B MACs, done on the host after gather.

GEMM1/GEMM2 run in fp8 e4m3 with perf_mode=DoubleRow: each PE cell holds two
weights (contraction 256 per pass), 2 fp8 MACs/cell/cycle => 2x the
bf16/fp32r rate. Weights are pre-scaled by 64 on the host so their ~0.02
magnitudes sit in e4m3's normal range; the 1/64 is folded into the ACT
engine's scale operand (out = func(in*scale + bias)), which also converts
the fp32 PSUM result straight to the fp8 tile feeding the next GEMM.
Accumulation is fp32 in PSUM, so the only precision loss is fp8 input
quantization; measured end-to-end rel err ~3e-3.

GEMM3 contracts H2=1024 down to a single output row, so running it on the
PE costs a full 1024-value stream per 512 batch columns (4 DoubleRow MMs /
chunk) for 0.05% of the FLOPs. Instead the idle Vector engine computes
acc[p,b] = sum_kb w3[kb*128+p] * h2[p,kb,b] (8 fused multiply-adds with a
per-partition scalar), and one ones.T @ acc fp32r matmul does the
128-partition reduction (216 ns/chunk instead of 864 ns). That matmul is
emitted two h-blocks into the NEXT chunk's GEMM1 so the in-order PE queue
never waits on the DVE. h2 stays fp32 and W3 unquantized on this path.
"""

import numpy as np
import ml_dtypes

import concourse.bass as bass
import concourse.bacc as bacc
import concourse.mybir as mybir
from concourse.bass_isa import ReduceOp
from concourse.bass_utils import run_bass_kernel_spmd
from concourse.tile import TileContext

E = 8
B = 16384
D = 1024
H = 2048
H2 = 1024
NB = 512  # batch columns per chunk (one PSUM bank of fp32)

F32 = mybir.dt.float32
F32R = mybir.dt.float32r
BF16 = mybir.dt.bfloat16
F8 = mybir.dt.float8e4
AF = mybir.ActivationFunctionType
ALU = mybir.AluOpType
DR = mybir.MatmulPerfMode.DoubleRow

DBLK = D // 128   # 8
HBLK = H // 128   # 16
KBLK = H2 // 128  # 8

WSCALE = 64.0  # host-side weight scale; inverted in the ACT ops

NP_F8 = ml_dtypes.float8_e4m3  # matches TRN FP8_EXP4 (max 240, inf beyond)


def build_moe_nc(batch: int = B) -> bass.Bass:
    nchunk = batch // NB
    nc = bacc.Bacc("TRN2")

    xT = nc.declare_dram_parameter("xT", [D, batch], F8, isOutput=False)
    w1 = nc.declare_dram_parameter("w1", [HBLK, 128, DBLK, 128], F8, isOutput=False)
    w2 = nc.declare_dram_parameter("w2", [H, H2], F8, isOutput=False)
    w3 = nc.declare_dram_parameter("w3", [128, KBLK], F32, isOutput=False)
    b1 = nc.declare_dram_parameter("b1", [128, HBLK], F32, isOutput=False)
    b2 = nc.declare_dram_parameter("b2", [128, KBLK], F32, isOutput=False)
    b3 = nc.declare_dram_parameter("b3", [1, 1], F32, isOutput=False)
    y = nc.declare_dram_parameter("y", [1, batch], F32, isOutput=True)

    inv = 1.0 / WSCALE

    with TileContext(nc) as tc:
        with (
            tc.tile_pool(name="wpool", bufs=1) as wpool,
            tc.tile_pool(name="xpool", bufs=2) as xpool,
            tc.tile_pool(name="hpool", bufs=2) as hpool,
            tc.tile_pool(name="apool", bufs=2) as apool,
            tc.tile_pool(name="ypool", bufs=4) as ypool,
            tc.tile_pool(name="pp1", bufs=4, space="PSUM") as pp1,
            tc.tile_pool(name="pp2", bufs=3, space="PSUM") as pp2,
            tc.tile_pool(name="ppw", bufs=1, space="PSUM") as ppw,
        ):
            # Weights resident in SBUF for the whole kernel. w1 arrives as 16
            # h-block slices so GEMM1 of chunk 0 can start once x(0) and the
            # first slice land instead of after the full tensor.
            w1_sb = wpool.tile([128, HBLK, DBLK, 128], F8)
            w3_sb = wpool.tile([128, KBLK], F32)
            nc.sync.dma_start(out=w3_sb, in_=w3[:, :])
            b1_sb = wpool.tile([128, HBLK], F32)
            nc.sync.dma_start(out=b1_sb, in_=b1[:, :])
            b2_sb = wpool.tile([128, KBLK], F32)
            nc.sync.dma_start(out=b2_sb, in_=b2[:, :])
            b3_sb = wpool.tile([1, 1], F32)
            nc.sync.dma_start(out=b3_sb, in_=b3[:, :])
            w2_sb = wpool.tile([128, HBLK, H2], F8)

            # HAM warm-up: the PE clock-gate needs ~3.4us of sustained matmul
            # activity to reach 2.4 GHz. The first real matmul can't issue
            # until x(0)+w1[0] land (~12us: framework preamble + DMA), so run
            # 16 junk DoubleRow matmuls on a memset tile during that window —
            # HAM is warm before real work starts and the junk finishes well
            # before the DMA does.
            warm_sb = wpool.tile([128, 2, NB], F8, name="warm_sb")
            nc.vector.memset(warm_sb, 0.0)
            ps_w = ppw.tile([128, NB], F32, name="ps_w")
            for _ in range(16):
                nc.tensor.matmul(
                    ps_w, warm_sb[:, :, 0:128], warm_sb, start=True, stop=True,
                    perf_mode=DR,
                )

            # Walrus allows only one semaphore wait per Matmult, and ACTIVATE
            # waits are precious too. These tiny "absorber" ops consume each
            # weight/bias DMA-done semaphore on the PE/ACT clocks so the first
            # real consumer needs at most one new wait.
            scratch = wpool.tile([1, 4], F32, name="scratch")

            def absorb_act(b_elem, i):
                nc.scalar.activation(scratch[0:1, i : i + 1], b_elem, AF.Copy)

            absorb_act(b1_sb[0:1, 0:1], 0)
            absorb_act(b2_sb[0:1, 0:1], 1)
            absorb_act(b3_sb[0:1, 0:1], 2)

            def absorb_pe(w_elem):
                # Standalone junk LDWEIGHTS (overwritten by the next real
                # matmul's own weight load) consumes a DMA-done semaphore on
                # the PE clock.
                nc.tensor.ldweights(w_elem)

            xT_r = xT.rearrange("(a p) (c n) -> p a c n", p=128, n=NB)

            # GEMM3 tail of chunk c, emitted during GEMM1 of chunk c+1 so the
            # in-order PE queue reaches the ones-matmul only after the DVE had
            # a couple of microseconds to finish acc.
            def emit_gemm3(pend):
                red, c = pend
                y_sb = ypool.tile([1, NB], F32, name="y_sb")
                nc.scalar.activation(
                    y_sb, red[0:1, :], AF.Sigmoid, bias=b3_sb[0:1, 0:1], scale=inv
                )
                nc.sync.dma_start(out=y[:, c * NB : (c + 1) * NB], in_=y_sb)

            pending = None
            for c in range(nchunk):
                x_sb = xpool.tile([128, DBLK, NB], F8, name="x_sb")
                if c == 0:
                    # First-MM dependencies first: w1 slice 0, then x(0) in
                    # d-pair slices (so GEMM1 h-block 0's t-th matmul waits
                    # only on pair t), then the rest of w1, then w2 (first
                    # needed by GEMM2).
                    nc.sync.dma_start(out=w1_sb[:, 0, :, :], in_=w1[0])
                    for t in range(DBLK // 2):
                        nc.sync.dma_start(
                            out=x_sb[:, 2 * t : 2 * t + 2, :],
                            in_=xT_r[:, 2 * t : 2 * t + 2, c, :],
                        )
                    for hb in range(1, HBLK):
                        nc.sync.dma_start(out=w1_sb[:, hb, :, :], in_=w1[hb])
                    w2_r = w2.rearrange("(a p) k -> p a k", p=128)
                    for hb in range(HBLK):
                        nc.sync.dma_start(
                            out=w2_sb[:, hb : hb + 1, :], in_=w2_r[:, hb : hb + 1, :]
                        )
                    absorb_pe(w1_sb[0:1, 0, 0, 0:1])
                else:
                    nc.sync.dma_start(out=x_sb, in_=xT_r[:, :, c, :])

                # GEMM1: h1T[h, b] = relu((W1s.T @ xT)/64 + b1), h on partitions.
                # DoubleRow: contract two 128-d blocks per pass.
                h1_sb = hpool.tile([128, HBLK, NB], F8, name="h1_sb")
                for hb in range(HBLK):
                    ps1 = pp1.tile([128, NB], F32, name="ps1")
                    for t in range(DBLK // 2):
                        nc.tensor.matmul(
                            ps1,
                            w1_sb[:, hb, 2 * t : 2 * t + 2, :],
                            x_sb[:, 2 * t : 2 * t + 2, :],
                            start=(t == 0),
                            stop=(t == DBLK // 2 - 1),
                            perf_mode=DR,
                        )
                    if hb == 2 and pending is not None:
                        emit_gemm3(pending)
                        pending = None
                    nc.scalar.activation(
                        h1_sb[:, hb, :], ps1, AF.Relu,
                        bias=b1_sb[:, hb : hb + 1], scale=inv,
                    )

                if c == 0:
                    absorb_pe(w2_sb[0:1, 0, 0:1])

                # GEMM2: h2T[k, b] = relu((W2s.T @ h1T)/64 + b2), k on
                # partitions, fp32 (consumed only by the DVE below).
                # GEMM3 partial on DVE, interleaved per k-block:
                # acc[p,b] = sum_kb w3[kb*128+p]*h2[p,kb,b] (w3 here is 64*W3,
                # so acc = 64 * z3 partials). fp32 ping-pong intermediates;
                # the last step rounds to bf16 for the fp-consistent matmul.
                h2_sb = hpool.tile([128, KBLK, NB], F32, name="h2_sb")
                acc_a = apool.tile([128, NB], F32, name="acc_a")
                acc_b = apool.tile([128, NB], F32, name="acc_b")
                acc_f = apool.tile([128, NB], F32, name="acc_f")
                src, dst = acc_a, acc_b
                for kb in range(KBLK):
                    ps2 = pp2.tile([128, NB], F32, name="ps2")
                    for t in range(HBLK // 2):
                        nc.tensor.matmul(
                            ps2,
                            w2_sb[:, 2 * t : 2 * t + 2, kb * 128 : (kb + 1) * 128],
                            h1_sb[:, 2 * t : 2 * t + 2, :],
                            start=(t == 0),
                            stop=(t == HBLK // 2 - 1),
                            perf_mode=DR,
                        )
                    nc.scalar.activation(
                        h2_sb[:, kb, :], ps2, AF.Relu,
                        bias=b2_sb[:, kb : kb + 1], scale=inv,
                    )
                    if kb == 0:
                        nc.vector.tensor_scalar_mul(
                            acc_a, h2_sb[:, 0, :], w3_sb[:, 0:1]
                        )
                    else:
                        out = acc_f if kb == KBLK - 1 else dst
                        nc.vector.scalar_tensor_tensor(
                            out, h2_sb[:, kb, :], w3_sb[:, kb : kb + 1], src,
                            ALU.mult, ALU.add,
                        )
                        src, dst = out, src
                # Partition reduction on the idle GpSimd engine (upcasts to
                # fp32); frees the PE of the former per-chunk ones-matmul.
                red = apool.tile([128, NB], F32, name="red")
                nc.gpsimd.partition_all_reduce(red, acc_f, 128, ReduceOp.add)
                pending = (red, c)

            emit_gemm3(pending)

    nc.finalize()
    return nc


def q8(a: np.ndarray) -> np.ndarray:
    """Quantize fp32 -> fp8 e4m3 (RNE, TRN-compatible in the +-240 range)."""
    return np.ascontiguousarray(np.asarray(a, dtype=np.float32)).astype(NP_F8)


def make_in_maps(
    x: np.ndarray,
    W1: np.ndarray,
    b1: np.ndarray,
    W2: np.ndarray,
    b2: np.ndarray,
    W3: np.ndarray,
    b3: np.ndarray,
) -> list[dict[str, np.ndarray]]:
    xT = np.ascontiguousarray(q8(x).T)
    in_maps = []
    for e in range(E):
        in_maps.append(
            {
                "xT": xT,
                "w1": np.ascontiguousarray(
                    q8(WSCALE * W1[e])
                    .reshape(DBLK, 128, HBLK, 128)
                    .transpose(2, 1, 0, 3)
                ),
                "w2": q8(WSCALE * W2[e]),
                "w3": np.ascontiguousarray(
                    (WSCALE * np.asarray(W3[e], dtype=np.float32))
                    .reshape(KBLK, 128)
                    .T
                ),
                "b1": np.ascontiguousarray(b1[e].reshape(HBLK, 128).T.astype(np.float32)),
                "b2": np.ascontiguousarray(b2[e].reshape(KBLK, 128).T.astype(np.float32)),
                "b3": np.asarray(b3[e], dtype=np.float32).reshape(1, 1),
            }
        )
    return in_maps


_NC_CACHE: dict[int, bass.Bass] = {}


def run_on_hw(in_maps, batch: int = B, **kw):
    nc = _NC_CACHE.get(batch)
    if nc is None:
        nc = build_moe_nc(batch)
        _NC_CACHE[batch] = nc
    return run_bass_kernel_spmd(nc, in_maps, list(range(E)), **kw)


def kernel(x, soft_cluster_probs, W1, b1, W2, b2, W3, b3) -> np.ndarray:
    in_maps = make_in_maps(x, W1, b1, W2, b2, W3, b3)
    res = run_on_hw(in_maps, batch=x.shape[0])
    y_all = np.stack([res.results[e]["y"][0] for e in range(E)], axis=0)  # [E, B]
    combined = np.einsum(
        "eb,be->b", y_all, np.asarray(soft_cluster_probs, dtype=np.float32)
    )
    return combined.astype(np.float32).reshape(-1, 1)


# revision 22
# speedup vs baseline: 1.0084x; 1.0084x over previous
"""Expert-parallel MoE (soft routing) kernel for 8 TRN2 NeuronCores.

Problem (nn_EnhancedMixtureOfExperts): every expert processes the full batch,
outputs mixed by soft cluster probabilities.

    h1 = relu(x @ W1[e] + b1[e])      x:[B,D]  W1[e]:[D,H]
    h2 = relu(h1 @ W2[e] + b2[e])     W2[e]:[H,H2]
    y  = sigmoid(h2 @ W3[e] + b3[e])  W3[e]:[H2,1]
    out[b] = sum_e y[e,b] * probs[b,e]

Sharding: expert-parallel — core e computes expert e over the full batch.
x is transposed on the host to xT [D,B] so on-chip activations are stored
feature-on-partition ([feat,128-block] x [batch]) and every GEMM consumes
weights in their natural [in,out] layout as the stationary operand.
The weighted combine is 8*B MACs, done on the host after gather.

GEMM1/GEMM2 run in fp8 e4m3 with perf_mode=DoubleRow: each PE cell holds two
weights (contraction 256 per pass), 2 fp8 MACs/cell/cycle => 2x the
bf16/fp32r rate. Weights are pre-scaled by 64 on the host so their ~0.02
magnitudes sit in e4m3's normal range; the 1/64 is folded into the ACT
engine's scale operand (out = func(in*scale + bias)), which also converts
the fp32 PSUM result straight to the fp8 tile feeding the next GEMM.
Accumulation is fp32 in PSUM, so the only precision loss is fp8 input
quantization; measured end-to-end rel err ~3e-3.

GEMM3 contracts H2=1024 down to a single output row, so running it on the
PE costs a full 1024-value stream per 512 batch columns (4 DoubleRow MMs /
chunk) for 0.05% of the FLOPs. Instead the idle Vector engine computes
acc[p,b] = sum_kb w3[kb*128+p] * h2[p,kb,b] (8 fused multiply-adds with a
per-partition scalar), and one ones.T @ acc fp32r matmul does the
128-partition reduction (216 ns/chunk instead of 864 ns). That matmul is
emitted two h-blocks into the NEXT chunk's GEMM1 so the in-order PE queue
never waits on the DVE. h2 stays fp32 and W3 unquantized on this path.
"""

import numpy as np
import ml_dtypes

import concourse.bass as bass
import concourse.bacc as bacc
import concourse.mybir as mybir
from concourse.bass_isa import ReduceOp
from concourse.bass_utils import run_bass_kernel_spmd
from concourse.tile import TileContext

E = 8
B = 16384
D = 1024
H = 2048
H2 = 1024
NB = 512  # batch columns per chunk (one PSUM bank of fp32)

F32 = mybir.dt.float32
F32R = mybir.dt.float32r
BF16 = mybir.dt.bfloat16
F8 = mybir.dt.float8e4
AF = mybir.ActivationFunctionType
ALU = mybir.AluOpType
DR = mybir.MatmulPerfMode.DoubleRow

DBLK = D // 128   # 8
HBLK = H // 128   # 16
KBLK = H2 // 128  # 8

WSCALE = 64.0  # host-side weight scale; inverted in the ACT ops

NP_F8 = ml_dtypes.float8_e4m3  # matches TRN FP8_EXP4 (max 240, inf beyond)


def build_moe_nc(batch: int = B) -> bass.Bass:
    nchunk = batch // NB
    nc = bacc.Bacc("TRN2")

    xT = nc.declare_dram_parameter("xT", [D, batch], F8, isOutput=False)
    w1 = nc.declare_dram_parameter("w1", [HBLK, 128, DBLK, 128], F8, isOutput=False)
    w2 = nc.declare_dram_parameter("w2", [H, H2], F8, isOutput=False)
    w3 = nc.declare_dram_parameter("w3", [128, KBLK], F32, isOutput=False)
    b1 = nc.declare_dram_parameter("b1", [128, HBLK], F32, isOutput=False)
    b2 = nc.declare_dram_parameter("b2", [128, KBLK], F32, isOutput=False)
    b3 = nc.declare_dram_parameter("b3", [1, 1], F32, isOutput=False)
    y = nc.declare_dram_parameter("y", [1, batch], F32, isOutput=True)

    inv = 1.0 / WSCALE

    with TileContext(nc) as tc:
        with (
            tc.tile_pool(name="wpool", bufs=1) as wpool,
            tc.tile_pool(name="xpool", bufs=2) as xpool,
            tc.tile_pool(name="hpool", bufs=2) as hpool,
            tc.tile_pool(name="apool", bufs=2) as apool,
            tc.tile_pool(name="ypool", bufs=4) as ypool,
            tc.tile_pool(name="pp1", bufs=3, space="PSUM") as pp1,
            tc.tile_pool(name="pp2", bufs=3, space="PSUM") as pp2,
            tc.tile_pool(name="pp3", bufs=1, space="PSUM") as pp3,
            tc.tile_pool(name="ppw", bufs=1, space="PSUM") as ppw,
        ):
            # Weights resident in SBUF for the whole kernel. w1 arrives as 16
            # h-block slices so GEMM1 of chunk 0 can start once x(0) and the
            # first slice land instead of after the full tensor.
            w1_sb = wpool.tile([128, HBLK, DBLK, 128], F8)
            w3_sb = wpool.tile([128, KBLK], F32)
            nc.sync.dma_start(out=w3_sb, in_=w3[:, :])
            b1_sb = wpool.tile([128, HBLK], F32)
            nc.sync.dma_start(out=b1_sb, in_=b1[:, :])
            b2_sb = wpool.tile([128, KBLK], F32)
            nc.sync.dma_start(out=b2_sb, in_=b2[:, :])
            b3_sb = wpool.tile([1, 1], F32)
            nc.sync.dma_start(out=b3_sb, in_=b3[:, :])
            w2_sb = wpool.tile([128, HBLK, H2], F8)

            # HAM warm-up: the PE clock-gate needs ~3.4us of sustained matmul
            # activity to reach 2.4 GHz. The first real matmul can't issue
            # until x(0)+w1[0] land (~12us: framework preamble + DMA), so run
            # 8 junk DoubleRow matmuls on a memset tile during that window —
            # HAM is warm before real work starts and the junk finishes well
            # before the DMA does.
            warm_sb = wpool.tile([128, 2, NB], F8, name="warm_sb")
            nc.vector.memset(warm_sb, 0.0)
            ones_sb = wpool.tile([128, 1], BF16, name="ones_sb")
            nc.vector.memset(ones_sb, 1.0)
            ps_w = ppw.tile([128, NB], F32, name="ps_w")
            for _ in range(8):
                nc.tensor.matmul(
                    ps_w, warm_sb[:, :, 0:128], warm_sb, start=True, stop=True,
                    perf_mode=DR,
                )

            # Walrus allows only one semaphore wait per Matmult, and ACTIVATE
            # waits are precious too. These tiny "absorber" ops consume each
            # weight/bias DMA-done semaphore on the PE/ACT clocks so the first
            # real consumer needs at most one new wait.
            scratch = wpool.tile([1, 4], F32, name="scratch")

            def absorb_act(b_elem, i):
                nc.scalar.activation(scratch[0:1, i : i + 1], b_elem, AF.Copy)

            absorb_act(b1_sb[0:1, 0:1], 0)
            absorb_act(b2_sb[0:1, 0:1], 1)
            absorb_act(b3_sb[0:1, 0:1], 2)

            def absorb_pe(w_elem):
                # Standalone junk LDWEIGHTS (overwritten by the next real
                # matmul's own weight load) consumes a DMA-done semaphore on
                # the PE clock.
                nc.tensor.ldweights(w_elem)

            xT_r = xT.rearrange("(a p) (c n) -> p a c n", p=128, n=NB)

            # GEMM3 tail of chunk c, emitted during GEMM1 of chunk c+1 so the
            # in-order PE queue reaches the ones-matmul only after the DVE had
            # a couple of microseconds to finish acc.
            def emit_gemm3(pend):
                acc, c = pend
                ps3 = pp3.tile([1, NB], F32, name="ps3")
                nc.tensor.matmul(ps3, ones_sb, acc, start=True, stop=True)
                y_sb = ypool.tile([1, NB], F32, name="y_sb")
                nc.scalar.activation(
                    y_sb, ps3, AF.Sigmoid, bias=b3_sb[0:1, 0:1], scale=inv
                )
                nc.sync.dma_start(out=y[:, c * NB : (c + 1) * NB], in_=y_sb)

            pending = None
            for c in range(nchunk):
                x_sb = xpool.tile([128, DBLK, NB], F8, name="x_sb")
                if c == 0:
                    # First-MM dependencies first: w1 slice 0, then x(0) in
                    # d-pair slices (so GEMM1 h-block 0's t-th matmul waits
                    # only on pair t), then the rest of w1, then w2 (first
                    # needed by GEMM2).
                    nc.sync.dma_start(out=w1_sb[:, 0, :, :], in_=w1[0])
                    for t in range(DBLK // 2):
                        nc.sync.dma_start(
                            out=x_sb[:, 2 * t : 2 * t + 2, :],
                            in_=xT_r[:, 2 * t : 2 * t + 2, c, :],
                        )
                    for hb in range(1, HBLK):
                        nc.sync.dma_start(out=w1_sb[:, hb, :, :], in_=w1[hb])
                    w2_r = w2.rearrange("(a p) k -> p a k", p=128)
                    for hb in range(HBLK):
                        nc.sync.dma_start(
                            out=w2_sb[:, hb : hb + 1, :], in_=w2_r[:, hb : hb + 1, :]
                        )
                    absorb_pe(w1_sb[0:1, 0, 0, 0:1])
                else:
                    nc.sync.dma_start(out=x_sb, in_=xT_r[:, :, c, :])

                # GEMM1: h1T[h, b] = relu((W1s.T @ xT)/64 + b1), h on partitions.
                # DoubleRow: contract two 128-d blocks per pass.
                h1_sb = hpool.tile([128, HBLK, NB], F8, name="h1_sb")
                for hb in range(HBLK):
                    ps1 = pp1.tile([128, NB], F32, name="ps1")
                    for t in range(DBLK // 2):
                        nc.tensor.matmul(
                            ps1,
                            w1_sb[:, hb, 2 * t : 2 * t + 2, :],
                            x_sb[:, 2 * t : 2 * t + 2, :],
                            start=(t == 0),
                            stop=(t == DBLK // 2 - 1),
                            perf_mode=DR,
                        )
                    if hb == 2 and pending is not None:
                        emit_gemm3(pending)
                        pending = None
                    nc.scalar.activation(
                        h1_sb[:, hb, :], ps1, AF.Relu,
                        bias=b1_sb[:, hb : hb + 1], scale=inv,
                    )

                if c == 0:
                    absorb_pe(w2_sb[0:1, 0, 0:1])

                # GEMM2: h2T[k, b] = relu((W2s.T @ h1T)/64 + b2), k on
                # partitions, fp32 (consumed only by the DVE below).
                # GEMM3 partial on DVE, interleaved per k-block:
                # acc[p,b] = sum_kb w3[kb*128+p]*h2[p,kb,b] (w3 here is 64*W3,
                # so acc = 64 * z3 partials). fp32 ping-pong intermediates;
                # the last step rounds to bf16 for the fp-consistent matmul.
                h2_sb = hpool.tile([128, KBLK, NB], F32, name="h2_sb")
                acc_a = apool.tile([128, NB], F32, name="acc_a")
                acc_b = apool.tile([128, NB], F32, name="acc_b")
                acc_f = apool.tile([128, NB], BF16, name="acc_f")
                src, dst = acc_a, acc_b
                for kb in range(KBLK):
                    ps2 = pp2.tile([128, NB], F32, name="ps2")
                    for t in range(HBLK // 2):
                        nc.tensor.matmul(
                            ps2,
                            w2_sb[:, 2 * t : 2 * t + 2, kb * 128 : (kb + 1) * 128],
                            h1_sb[:, 2 * t : 2 * t + 2, :],
                            start=(t == 0),
                            stop=(t == HBLK // 2 - 1),
                            perf_mode=DR,
                        )
                    nc.scalar.activation(
                        h2_sb[:, kb, :], ps2, AF.Relu,
                        bias=b2_sb[:, kb : kb + 1], scale=inv,
                    )
                    if kb == 0:
                        nc.vector.tensor_scalar_mul(
                            acc_a, h2_sb[:, 0, :], w3_sb[:, 0:1]
                        )
                    else:
                        out = acc_f if kb == KBLK - 1 else dst
                        nc.vector.scalar_tensor_tensor(
                            out, h2_sb[:, kb, :], w3_sb[:, kb : kb + 1], src,
                            ALU.mult, ALU.add,
                        )
                        src, dst = out, src
                pending = (acc_f, c)

            emit_gemm3(pending)

    nc.finalize()
    return nc


def q8(a: np.ndarray) -> np.ndarray:
    """Quantize fp32 -> fp8 e4m3 (RNE, TRN-compatible in the +-240 range)."""
    return np.ascontiguousarray(np.asarray(a, dtype=np.float32)).astype(NP_F8)


def make_in_maps(
    x: np.ndarray,
    W1: np.ndarray,
    b1: np.ndarray,
    W2: np.ndarray,
    b2: np.ndarray,
    W3: np.ndarray,
    b3: np.ndarray,
) -> list[dict[str, np.ndarray]]:
    xT = np.ascontiguousarray(q8(x).T)
    in_maps = []
    for e in range(E):
        in_maps.append(
            {
                "xT": xT,
                "w1": np.ascontiguousarray(
                    q8(WSCALE * W1[e])
                    .reshape(DBLK, 128, HBLK, 128)
                    .transpose(2, 1, 0, 3)
                ),
                "w2": q8(WSCALE * W2[e]),
                "w3": np.ascontiguousarray(
                    (WSCALE * np.asarray(W3[e], dtype=np.float32))
                    .reshape(KBLK, 128)
                    .T
                ),
                "b1": np.ascontiguousarray(b1[e].reshape(HBLK, 128).T.astype(np.float32)),
                "b2": np.ascontiguousarray(b2[e].reshape(KBLK, 128).T.astype(np.float32)),
                "b3": np.asarray(b3[e], dtype=np.float32).reshape(1, 1),
            }
        )
    return in_maps


_NC_CACHE: dict[int, bass.Bass] = {}


def run_on_hw(in_maps, batch: int = B, **kw):
    nc = _NC_CACHE.get(batch)
    if nc is None:
        nc = build_moe_nc(batch)
        _NC_CACHE[batch] = nc
    return run_bass_kernel_spmd(nc, in_maps, list(range(E)), **kw)


def kernel(x, soft_cluster_probs, W1, b1, W2, b2, W3, b3) -> np.ndarray:
    in_maps = make_in_maps(x, W1, b1, W2, b2, W3, b3)
    res = run_on_hw(in_maps, batch=x.shape[0])
    y_all = np.stack([res.results[e]["y"][0] for e in range(E)], axis=0)  # [E, B]
    combined = np.einsum(
        "eb,be->b", y_all, np.asarray(soft_cluster_probs, dtype=np.float32)
    )
    return combined.astype(np.float32).reshape(-1, 1)
